# revision 19
# baseline (speedup 1.0000x reference)
"""GraphVAE MPM kernel for Trainium2 (Bass/Tile), self-contained.

Math: the reference's S[i,j,a,b] tensor is separable off the overrides:
S = c_ij * Q[a,b] with c in {0,1}, so the per-iteration O(N^4) masked
max-product collapses to an O(N^3) grouped max (T1[j,a] = max_b Qz[a,b]*X[j,b],
clamped by G[j] = -1e6*min_{b>=R} X[j,b]) plus a 64x64 matmul with Cz.
Edge terms outside the real-node block reduce to per-row scalars built from
G via masked partition-sums (done as PE matmuls against static 0/1 matrices).

Key structural optimizations:
- The whole iteration map is positively 1-homogeneous in X, so the exact L2
  normalization can be replaced in-loop by ANY positive per-iteration scale
  (one exact normalize after the loop reproduces the reference bit-for-bit
  up to fp noise).  The scale used is 1/|sum_j G[j]| -- a magnitude proxy
  that reuses the already-needed Sg partition-sum, so the entire norm
  pipeline (square/reduce/rsqrt) disappears from the loop.  Carried
  magnitudes stay in [1e-2, 1e6] (fp32-safe; verified over 20 iterations).
- The big O(N^3) product+max runs on ALL 128 DVE partitions: partition
  p = j + 64*h covers a-half h, with X duplicated to both halves via a PE
  matmul against a stacked-identity matrix.  The product runs in bf16
  (2x DVE mode); a bf16 tensor_tensor max pre-folds the b-axis in half
  (2x) before the 1x tensor_reduce.  The G clamp rides in a trailing
  column of the folded tensor so the reduce applies it for free.
- The tail block (columns >= R) and all per-row scalars are carried
  DUPLICATED on all 128 partitions, so the clamp chain
  (tmin -> G -> clamp column) never crosses partitions and hides entirely
  under the big DVE ops.
- Small ops are spread over ACT/PE/GPSIMD; no ACT function outside the
  exp_and_others table set is used inside the loop (a Sqrt would force a
  ~2.7us ACT table reload per iteration; the one true Sqrt runs once after
  the loop).
- The 20 iterations run as a For_i hardware loop (program size constant in
  the trip count -- also what makes the iteration-slope timing in test.py
  meaningful), fully unrolled inside one loop body per back-edge.

The same program is replicated SPMD on all 8 cores (the problem is a single
small graph; there is no profitable cross-core split of a 64-node MPM whose
state fits in one core's SBUF many times over) and core 0's output is
returned.
"""

import numpy as np

N = 64
R = 56
H = R // 2          # 28: a-half per partition group
W = R + 1           # 57: b-columns incl. the G clamp column
ITERS = 20
UNROLL = 20
BIGNEG = -3.0e38
RSQRT_MAGIC = 0x5F3759DF
USE_BF16 = True

_CACHE = {}


def _precompute(A_gt, vec_logits):
    """Host-side O(N^2) constant construction (mirrors reference's setup)."""
    import ml_dtypes

    bf16 = ml_dtypes.bfloat16 if USE_BF16 else np.float32
    A_gt = np.asarray(A_gt, np.float32)
    vec = np.asarray(vec_logits, np.float32)
    d = np.arange(N)

    iu = np.triu_indices(N, k=1)
    logits = np.zeros((N, N), np.float32)
    logits[iu] = vec
    logits = logits + logits.T
    logits[d, d] = np.float32(-10.0)
    B = (1.0 / (1.0 + np.exp(-logits))).astype(np.float32)

    A = A_gt.copy()
    r = int((A.sum(1) > 0).sum())
    real = d < r
    A[d, d] = np.where(real, np.float32(1.0), A[d, d])
    Bm = B.copy()
    Bm[d, d] = np.where(real, np.float32(1.0), Bm[d, d])
    dA = np.diagonal(A).copy()
    dB = np.diagonal(Bm).copy()
    degA = A.sum(1)
    degB = Bm.sum(1)
    node_sim = (1.0 / (np.abs(degA[:, None] - degB[None, :]) + 1.0)).astype(np.float32)

    Qz = (Bm * dB[:, None] * dB[None, :]).astype(np.float32)
    np.fill_diagonal(Qz, 0.0)

    # [128, H, W]: partition p=(h*64+j) holds Qz[28h+a', b] (j-independent),
    # with a zero in the trailing clamp column.
    qz2 = np.zeros((128, H, W), np.float32)
    for h in range(2):
        qz2[64 * h:64 * (h + 1), :, :R] = Qz[28 * h:28 * (h + 1), :R][None, :, :]

    Cz = (A * dA[:, None] * dA[None, :]).astype(np.float32)
    np.fill_diagonal(Cz, 0.0)
    Cz[:, R:] = 0.0
    Cz[R:, :] = 0.0
    # [128, 128]: cols 0:64 = [Cz; 0], cols 64:128 = [0; Cz] — K=128 weight
    # matrices so the m matmuls read rhs t1 with no partition offset.
    czd = np.zeros((128, 2 * N), np.float32)
    czd[0:N, 0:N] = Cz
    czd[N:128, N:2 * N] = Cz

    dup = np.zeros((N, 128), np.float32)    # stacked identity: dup@v duplicates
    dup[d, d] = 1.0
    dup[d, d + 64] = 1.0

    ns = (dA[:, None] * dB[None, :] * node_sim).astype(np.float32)
    mask2 = (d[:, None] < R) & (d[None, :] < R)
    nsm = np.where(mask2, ns, np.float32(-1e6)).astype(np.float32)

    pv = np.zeros((128, 2), np.float32)
    pv[R:N, 0] = BIGNEG
    pv[N + R:, 0] = BIGNEG

    return {
        "qz2p": qz2.reshape(128, H * W).astype(bf16),
        "czdp": czd.astype(bf16),
        "dupp": dup.astype(np.float32),
        "czp": Cz.astype(np.float32),
        "nsmp": nsm.astype(np.float32),
        "pvp": pv.astype(np.float32),
    }


def _build(iters=ITERS):
    import concourse.bass as bass
    import concourse.mybir as mybir
    from concourse import bacc
    from concourse.tile import TileContext

    assert iters % UNROLL == 0
    f32 = mybir.dt.float32
    bf = mybir.dt.bfloat16 if USE_BF16 else mybir.dt.float32
    i32 = mybir.dt.int32
    ALU = mybir.AluOpType
    ACTF = mybir.ActivationFunctionType
    AX = mybir.AxisListType

    nc = bacc.Bacc()
    qz2p = nc.declare_dram_parameter("qz2p", [128, H * W], bf, isOutput=False)
    czdp = nc.declare_dram_parameter("czdp", [128, 2 * N], bf, isOutput=False)
    dupp = nc.declare_dram_parameter("dupp", [N, 128], f32, isOutput=False)
    czp = nc.declare_dram_parameter("czp", [N, N], f32, isOutput=False)
    nsmp = nc.declare_dram_parameter("nsmp", [N, N], f32, isOutput=False)
    pvp = nc.declare_dram_parameter("pvp", [128, 2], f32, isOutput=False)
    xoutp = nc.declare_dram_parameter("xout", [N, N], f32, isOutput=True)

    with TileContext(nc) as tc:
        with (
            tc.tile_pool(name="consts", bufs=1) as cp,
            tc.tile_pool(name="big", bufs=2) as bp,
            tc.tile_pool(name="sm", bufs=2) as sp,
            tc.tile_pool(name="ps", bufs=2, space="PSUM") as pp,
        ):
            # ---- constants ----
            qz2 = cp.tile([128, H * W], bf, name="qz2")
            nc.sync.dma_start(out=qz2, in_=qz2p[:])
            czd = cp.tile([128, 2 * N], bf, name="czd")
            nc.sync.dma_start(out=czd, in_=czdp[:])
            dup = cp.tile([N, 128], f32, name="dup")
            nc.sync.dma_start(out=dup, in_=dupp[:])
            cz = cp.tile([N, N], f32, name="cz")
            nc.sync.dma_start(out=cz, in_=czp[:])
            nsm = cp.tile([N, N], f32, name="nsm")
            nc.sync.dma_start(out=nsm, in_=nsmp[:])
            pv = cp.tile([128, 2], f32, name="pv")
            nc.sync.dma_start(out=pv, in_=pvp[:])

            ones128 = cp.tile([N, 128], f32, name="ones128")
            nc.vector.memset(ones128, 1.0)
            onesb = cp.tile([128, H], f32, name="onesb")
            nc.vector.memset(onesb, 1.0)
            selin = cp.tile([N, N], f32, name="selin")
            nc.vector.memset(selin, 0.0)
            nc.vector.memset(selin[:, 0:R], 1.0)
            selout = cp.tile([N, N], f32, name="selout")
            nc.vector.memset(selout, 0.0)
            nc.vector.memset(selout[:, R:N], 1.0)

            # loop-carried state (tail + per-row scalars duplicated on all
            # 128 partitions so the clamp chain never crosses partitions)
            x = cp.tile([N, R], f32, name="x0")
            nc.vector.memset(x, 1.0 / N)
            xt2 = cp.tile([128, N - R], f32, name="xt2")
            nc.vector.memset(xt2, 1.0 / N)
            x2 = cp.tile([128, R], bf, name="x2")
            nc.vector.memset(x2, 1.0 / N)
            tmin2 = cp.tile([128, 1], f32, name="tmin0")
            nc.vector.memset(tmin2, 1.0 / N)


            qz2v = bass.AP(tensor=qz2.tensor, offset=qz2.offset,
                           ap=[list(qz2.ap[0]), [W, H], [1, R]])

            def body():
                # --- per-row scalar chain (ACT/GPSIMD/PE) ---
                g = sp.tile([128, 1], f32, tag="g", name="g")
                nc.scalar.activation(g, tmin2, ACTF.Copy, bias=0.0, scale=-1.0e6)
                gn = sp.tile([128, 1], f32, tag="gn", name="gn")
                nc.scalar.activation(gn, tmin2, ACTF.Copy, bias=0.0, scale=1.0e6)
                t0 = sp.tile([128, 1], f32, tag="t0", name="t0")
                nc.scalar.activation(t0, g, ACTF.Relu, bias=0.0, scale=1.0)
                tsel = sp.tile([N, 1], f32, tag="tsel", name="tsel")
                nc.gpsimd.tensor_scalar(tsel, g[0:N, 0:1], pv[0:N, 0:1], -1.0,
                                        ALU.max, ALU.mult)

                psc = pp.tile([128, 8], f32, tag="psc", name="psc")
                nc.tensor.matmul(psc[0:N, 0:1], selin, t0[0:N, 0:1],
                                 start=True, stop=False)
                nc.tensor.matmul(psc[0:N, 0:1], selout, g[0:N, 0:1],
                                 start=False, stop=True)
                nc.tensor.matmul(psc[:, 1:2], ones128, g[0:N, 0:1],
                                 start=True, stop=True)
                nc.tensor.matmul(psc[0:N, 2:3], cz, t0[0:N, 0:1],
                                 start=True, stop=True)

                ag = sp.tile([128, 1], f32, tag="ag", name="ag")
                nc.scalar.activation(ag, psc[:, 1:2], ACTF.Abs,
                                     bias=0.0, scale=1.0)
                invn2 = sp.tile([128, 1], f32, tag="invn", name="invn")
                nc.vector.reciprocal(invn2, ag)
                invn = invn2[0:N, 0:1]

                e_sel = sp.tile([N, 1], f32, tag="e_sel", name="esel")
                nc.scalar.activation(e_sel, psc[0:N, 0:1], ACTF.Identity,
                                     bias=tsel, scale=1.0)
                esel2 = sp.tile([N, 1], f32, tag="esel2", name="esel2")
                nc.scalar.activation(esel2, psc[0:N, 2:3], ACTF.Identity,
                                     bias=e_sel, scale=-1.0)
                e_tail2 = sp.tile([128, 1], f32, tag="e_tail", name="etail")
                nc.scalar.activation(e_tail2, psc[:, 1:2], ACTF.Identity,
                                     bias=gn, scale=1.0)

                # --- big ops (DVE, 128 partitions, bf16 product) ---
                u = bp.tile([128, H * R], bf, tag="u", name="u")
                u_main = bass.AP(tensor=u.tensor, offset=u.offset,
                                 ap=[list(u.ap[0]), [R, H], [1, R]])
                x2b = bass.AP(tensor=x2.tensor, offset=x2.offset,
                              ap=[list(x2.ap[0]), [0, H], [1, R]])
                nc.vector.tensor_tensor(u_main, x2b, qz2v, ALU.mult)
                HB = R // 2  # 28: b-fold width
                v = bp.tile([128, H * (HB + 1)], bf, tag="v", name="v")
                v_gcol = bass.AP(tensor=v.tensor, offset=v.offset + HB,
                                 ap=[list(v.ap[0]), [HB + 1, H]])
                nc.scalar.activation(v_gcol, onesb, ACTF.Identity,
                                     bias=g, scale=0.0)
                v_main = bass.AP(tensor=v.tensor, offset=v.offset,
                                 ap=[list(v.ap[0]), [HB + 1, H], [1, HB]])
                u_lo = bass.AP(tensor=u.tensor, offset=u.offset,
                               ap=[list(u.ap[0]), [R, H], [1, HB]])
                u_hi = bass.AP(tensor=u.tensor, offset=u.offset + HB,
                               ap=[list(u.ap[0]), [R, H], [1, HB]])
                nc.vector.tensor_tensor(v_main, u_lo, u_hi, ALU.max)
                v_all = bass.AP(tensor=v.tensor, offset=v.offset,
                                ap=[list(v.ap[0]), [HB + 1, H], [1, HB + 1]])
                t1 = sp.tile([128, H], bf, tag="t1", name="t1")
                nc.vector.tensor_reduce(t1, v_all, AX.X, ALU.max)

                m = pp.tile([N, R], f32, tag="m", name="m")
                nc.tensor.matmul(m[:, 0:H], czd[:, 0:N], t1,
                                 start=True, stop=True)
                nc.tensor.matmul(m[:, H:R], czd[:, N:2 * N], t1,
                                 start=True, stop=True)

                # --- assemble the new x directly with the LAGGED scale
                # (invn from the previous body; norm is taken on the SCALED
                # x, which is equally valid by homogeneity and keeps the
                # carried magnitude near-constant) ---
                esel3 = sp.tile([N, 1], f32, tag="esel3", name="esel3")
                nc.gpsimd.tensor_tensor(esel3, esel2, invn, ALU.mult)
                xna = sp.tile([N, R], f32, tag="xna", name="xna")
                nc.gpsimd.tensor_tensor(xna, x, nsm[:, 0:R], ALU.mult)
                xna2 = sp.tile([N, R], f32, tag="xna2", name="xna2")
                nc.gpsimd.tensor_scalar(xna2, xna, invn, esel3,
                                        ALU.mult, ALU.add)
                xnt2 = sp.tile([128, N - R], f32, tag="xnt", name="xnt")
                nc.scalar.activation(xnt2, xt2, ACTF.Identity,
                                     bias=e_tail2, scale=-1.0e6)

                # x_in = m*invn + xna2  (single fused DVE op)
                nc.vector.scalar_tensor_tensor(x, m, invn, xna2,
                                               ALU.mult, ALU.add)
                nc.vector.tensor_scalar(xt2, xnt2, invn2,
                                        None, ALU.mult, ALU.min,
                                        accum_out=tmin2)
                xdp = pp.tile([128, R], f32, tag="xdp", name="xdp")
                nc.tensor.matmul(xdp, dup, x, start=True, stop=True)
                nc.scalar.activation(x2, xdp, ACTF.Copy, bias=0.0, scale=1.0)


            with tc.For_i(0, iters // UNROLL, 1):
                for _ in range(UNROLL):
                    body()

            # --- final exact normalization (homogeneity: one true L2 norm) ---
            scrf = sp.tile([N, R], f32, tag="scr", name="scrf")
            qrowf = sp.tile([N, 1], f32, tag="qrow", name="qrowf")
            nc.scalar.activation(scrf, x, ACTF.Square, bias=0.0, scale=1.0,
                                 accum_out=qrowf)
            scrft = sp.tile([N, N - R], f32, tag="scrt", name="scrft")
            qrowft = sp.tile([N, 1], f32, tag="qrow2", name="qrowft")
            nc.scalar.activation(scrft, xt2[0:N, :], ACTF.Square, bias=0.0,
                                 scale=1.0, accum_out=qrowft)
            npf = pp.tile([N, 1], f32, tag="npf", name="npf")
            nc.tensor.matmul(npf, ones128[:, 0:N], qrowf, start=True, stop=False)
            nc.tensor.matmul(npf, ones128[:, 0:N], qrowft, start=False, stop=True)
            snf = sp.tile([N, 1], f32, tag="snf", name="snf")
            nc.scalar.activation(snf, npf, ACTF.Sqrt, bias=0.0, scale=1.0)
            invf = sp.tile([N, 1], f32, tag="invf", name="invf")
            nc.vector.reciprocal(invf, snf)
            xo = sp.tile([N, N], f32, tag="xo", name="xo")
            nc.vector.tensor_scalar(xo[:, 0:R], x, invf, None, ALU.mult)
            nc.vector.tensor_scalar(xo[:, R:N], xt2[0:N, :], invf, None,
                                    ALU.mult)
            nc.sync.dma_start(out=xoutp[:], in_=xo)

    nc.finalize()
    return nc


def _get_nc(iters=ITERS):
    key = ("nc", iters)
    if key not in _CACHE:
        _CACHE[key] = _build(iters)
    return _CACHE[key]


def kernel(A_gt, vec_logits, R_int):
    assert int(R_int) == R and A_gt.shape == (N, N)
    ins = _precompute(A_gt, vec_logits)
    nc = _get_nc()

    from concourse.bass_utils import run_bass_kernel_spmd

    core_ids = list(range(8))
    res = run_bass_kernel_spmd(nc, [dict(ins) for _ in core_ids], core_ids)
    out = np.asarray(res.results[0]["xout"], dtype=np.float32).reshape(N, N)
    return out
